# revision 4
# baseline (speedup 1.0000x reference)
"""Trainium2 Bass kernel for nn_DotProductAttention_6030134084023.

reference: softmax(mask(Q @ K^T / sqrt(64), valid_lens)) @ V
  query/key/value: [64, 1024, 64] f32, valid_lens: [64] int32 -> [64, 1024, 64] f32

Strategy (v3)
-------------
Batch dim sharded across the 8 NeuronCores; host sorts batches by valid_len
(descending) and deals them round-robin so slot s holds similar lengths on
every core. Compiled per valid_lens chunk-count pattern.

All matmuls bf16 (fp32 PSUM); host pre-casts/transposes and duplicates Q/K
across both SBUF partition halves (the ST pair packing reads chunk A from
partitions 0:64 and chunk B from 64:128). Masking is folded into V on the
host: V rows (and the appended ones-columns) are zeroed for keys >= vl, so
invalid keys contribute exactly 0 to both numerator and denominator -- the
kernel computes exp with NO mask/bias anywhere, which lets exp ops fuse
freely across chunk/slot boundaries.

Each slot is processed in two query passes (h0: q 0:512, h1: 512:1024) so
the UT accumulator is [128,512] = 1 PSUM bank (utp bufs=2 -> no stall at
slot/pass boundaries). The work stream is a flat sequence of 512-col
"blocks" (slot, pass, chunk); blocks land in [128,1536] 3-bank PSUM super-
tiles (stp bufs=2; 6 banks; 8 total with utp):

  ST block  = K_c^T @ Q_h          PE bf16; chunk pairs concurrent via
                                   tile_position row packing (K=64 each)
  exp tile  = exp(0.125 * ST)      ScalarE, ONE fused [128,<=1536] ACTIVATE
                                   per super-tile (3 blocks share the
                                   352-cycle instruction overhead)
  UT_bh    += Vm_c^T @ EST_block   PE bf16 K=128, PSUM-accumulated; Vm_c =
                                   [V_c | ones] so UT rows 64:128 hold the
                                   denominator replicated (lane-local
                                   normalize); UTs deferred one super-tile
                                   to avoid head-of-line blocking STs

The ACT exp table is preloaded at t=0 by a dummy ACTIVATE so the ~1.3us
table load overlaps the input DMA wait. Slot 0's K (first 2 chunks) and
Q h0 are DMA'd first so the pipeline ramps early. Q/K ride the sync DMA
queue; V/den/out ride gpsimd.

Postprocess per (slot, pass): DVE copy UT PSUM->SBUF, DMA the denominator
rows from partitions 64:128 to 0:64 (engines are partition-local; DMA is
the only cross-partition mover), fused custom-DVE osb = num * approx(1/den)
(bit-trick seed + one Newton step), bf16 out DMA. The final pass skips the
numerator copy (NRECIP reads it straight from PSUM) and copies only the
denominator rows, shortening the kernel tail.
"""

import re

import numpy as np
import ml_dtypes

import concourse.bass as bass
import concourse.bacc as bacc
import concourse.tile as tile
from concourse import mybir
from concourse import bass_utils
from concourse import dve_ops as _dve_ops
from concourse.dve_ops import DveOp
from concourse.dve_spec import Spec, Src0, Src1, One, C0, C1, C2, sq

F32 = mybir.dt.float32
BF16 = mybir.dt.bfloat16
I32 = mybir.dt.int32
AF = mybir.ActivationFunctionType
ALU = mybir.AluOpType

NCORES = 8
B = 64
S = 1024
D = 64
BPC = B // NCORES  # 8 batch slots per core
KC = S // 128  # 8 k-chunks of 128
QH = 512  # q-half (one pass)
TB = 3  # ST blocks per PSUM super-tile (3 banks)

DVE_EXP_MOD = 0  # DVE-exp offload disabled for now

_BUILD_CACHE = {}


# ---------------------------------------------------------------------------
# Custom DVE ops (exp offload + fused reciprocal-multiply)
# ---------------------------------------------------------------------------

def _register_op(name, spec):
    """Register a new custom-DVE op at runtime: reserve a free opcode row,
    then compile once to learn (and pin) the uops sha."""
    from concourse.dve_table_gen import free_opcode_rows

    if name in _dve_ops._SUB_OPCODE_FOR_NAME:
        return next(o for o in _dve_ops.OPS if o.name == name)
    row = _dve_ops._CUSTOM_DVE_ROW_BASE + len(_dve_ops.OPS)
    if row >= 0x20 or row not in set(free_opcode_rows("TRN2")):
        raise RuntimeError(f"no free custom-DVE row for {name}")
    op = DveOp(name, spec, subdim=False, uops_sha={})
    _dve_ops.OPS.append(op)
    _dve_ops._SUB_OPCODE_FOR_NAME[name] = row
    try:
        op.compile("v3")
    except ValueError as e:
        m = re.search(r"v3: ([0-9a-f]+)", str(e))
        if not m:
            _dve_ops.OPS.remove(op)
            del _dve_ops._SUB_OPCODE_FOR_NAME[name]
            raise
        op = DveOp(name, spec, subdim=False, uops_sha={"v3": m.group(1)})
        _dve_ops.OPS[-1] = op
    op.compile("v3")
    _dve_ops.CUSTOM_DVE_SPECS[name] = op.spec
    return op


def _bitnot(x):
    from concourse.dve_spec import Bin, AluOp

    return Bin(AluOp.BITWISE_NOT, x, x)


def _np_bitnot(x):
    return (~np.ascontiguousarray(x, np.float32).view(np.int32)).view(np.float32)


# seed constants shared with RECIPROCAL_APPROX_FAST (1-NR variant: ~0.17% err)
_RC0, _RC1 = -0.23549792, 2.0017324

ALPHA = 0.125 / 256.0  # exp arg prescale for the DVE poly path (8 squarings)


def _register_dve_ops():
    try:
        # est = ((b2*s + b1)*s + 1) * mask   (deg-2 poly of exp(s/2048))
        op1 = _register_op(
            "EXP_P2M_ANT",
            Spec(
                body=((C1 * Src0 + C2) * Src0 + One) * C0,
                reference=lambda in0, in1, s0, s1, imm2: (
                    (s1 * in0 + imm2) * in0 + 1.0
                )
                * s0,
            ),
        )
        sq8 = Src0
        for _ in range(8):
            sq8 = sq(sq8)
        op2 = _register_op(
            "EXP_SQ8_ANT",
            Spec(body=sq8, reference=lambda in0, in1, s0, s1, imm2: in0 ** 256),
        )
        # out = in1 / in0 (approx): bit-trick seed + one Newton pass, fused mult
        _y0 = _bitnot(Src0) * C0
        op3 = _register_op(
            "NRECIP_MUL_ANT",
            Spec(
                body=(_y0 * (C1 - Src0 * _y0)) * Src1,
                reference=lambda in0, in1, s0, s1, imm2: (
                    (_np_bitnot(in0) * s0)
                    * (s1 - in0 * (_np_bitnot(in0) * s0))
                )
                * in1,
            ),
        )
        return op1, op2, op3
    except Exception:
        return None


_DVE_OPS = _register_dve_ops()
_HAVE_DVE_EXP = _DVE_OPS is not None
if _DVE_OPS:
    _EXP_P2M, _EXP_SQ8, _NRECIP_MUL = _DVE_OPS
_B1 = ALPHA
_B2 = ALPHA * ALPHA / 2.0


def _build(nprocs, nreals, debug_dump=False, ncores=NCORES):
    """nreals[s]: number of 128-key chunks with any valid key for slot s."""
    nc = bacc.Bacc("TRN2", target_bir_lowering=False, debug=False, num_devices=ncores)
    # host duplicates Q/K across both partition halves: [BPC, 128, S]
    qt = nc.dram_tensor("qt", [BPC, 128, S], BF16, kind="ExternalInput").ap()
    kt = nc.dram_tensor("kt", [BPC, 128, S], BF16, kind="ExternalInput").ap()
    # v is [V | ones] with rows >= vl zeroed (masking folded into V)
    v = nc.dram_tensor("v", [BPC, S, 128], BF16, kind="ExternalInput").ap()
    ot = nc.dram_tensor("ot", [BPC, D, S], BF16, kind="ExternalOutput").ap()

    # Longest slot FIRST: the pipeline ramp needs deep ST lookahead before
    # PE hits exp-dependent UT matmuls.
    slot_order = [0, 6, 7, 1, 4, 2, 5, 3]

    # flat block stream: (slot, pass, chunk); pairs of chunks pack on PE
    blocks = []  # (b, h, c)
    for b in slot_order:
        for h in (0, 1):
            for c in range(nreals[b]):
                blocks.append((b, h, c))
    nblocks = len(blocks)
    ntiles = -(-nblocks // TB)

    with tile.TileContext(nc) as tc:
        with (
            tc.tile_pool(name="const", bufs=1) as constp,
            tc.tile_pool(name="est", bufs=5) as estp,
            tc.tile_pool(name="usb", bufs=4) as usbp,
            tc.tile_pool(name="osb", bufs=3) as osbp,
            tc.tile_pool(name="stp", bufs=2, space="PSUM") as stp,
            tc.tile_pool(name="utp", bufs=2, space="PSUM") as utp,
        ):
            # ---- ACT exp-table preload: dummy ACTIVATE with no DMA deps ----
            warm = constp.tile([128, 1], F32, tag="warm")
            nc.gpsimd.memset(warm[:], 0.0)
            wout = constp.tile([128, 1], F32, tag="wout")
            nc.scalar.activation(out=wout[:], in_=warm[:], func=AF.Exp, scale=0.125)

            # ---- persistent SBUF inputs ----
            qt2 = constp.tile([128, BPC * S], BF16, tag="qt2")
            kt2 = constp.tile([128, BPC * S], BF16, tag="kt2")
            # vma[p, (b*KC + kc)*128 + j]: j<64 -> V[b, kc*128+p, j]; j>=64 ->
            # 1.0 (or 0.0 where kc*128+p >= vl[b])
            vma = constp.tile([128, BPC * KC * 128], BF16, tag="vma")
            vview = vma[:].rearrange("p (b j) -> p b j", j=128)

            # Q/K on the sync queue, V on gpsimd, slot-priority groups.
            # Slot 0's first K chunks + Q h0 land first so STs start early.
            dma_groups = [slot_order[0:1]] + [
                slot_order[1 + 2 * g : 3 + 2 * g] for g in range(4)
            ]
            for gi, grp in enumerate(dma_groups):
                for b in grp:
                    kwb = nreals[b] * 128
                    if gi == 0:
                        ksp = min(256, kwb)
                        nc.sync.dma_start(
                            out=kt2[:, b * S : b * S + ksp], in_=kt[b, :, 0:ksp]
                        )
                        nc.sync.dma_start(
                            out=qt2[:, b * S : b * S + QH], in_=qt[b, :, 0:QH]
                        )
                        nc.sync.dma_start(
                            out=qt2[:, b * S + QH : b * S + S], in_=qt[b, :, QH:S]
                        )
                        if kwb > ksp:
                            nc.sync.dma_start(
                                out=kt2[:, b * S + ksp : b * S + kwb],
                                in_=kt[b, :, ksp:kwb],
                            )
                    else:
                        nc.sync.dma_start(
                            out=qt2[:, b * S : (b + 1) * S], in_=qt[b]
                        )
                        nc.sync.dma_start(
                            out=kt2[:, b * S : b * S + kwb], in_=kt[b, :, 0:kwb]
                        )
                    nc.gpsimd.dma_start(
                        out=vview[:, b * KC : b * KC + nreals[b], :],
                        in_=bass.AP(
                            tensor=v.tensor,
                            offset=v[b].offset,
                            ap=[[128, 128], [128 * 128, nreals[b]], [1, 128]],
                        ),
                    )

            # ---- main stream ----
            st_tiles = {}  # tile idx -> psum tile
            est_tiles = {}  # tile idx -> est sbuf tile
            ut_tiles = {}  # (b, h) -> psum tile
            block_ref = {}  # block idx -> (tile idx, col)
            ready_by_tile = {}  # tile idx -> [block idx] fully est'd
            last_done_tile = [-1]

            def emit_exp(t, fill):
                est_t = estp.tile([128, TB * QH], BF16, tag="est", name="est")
                est_tiles[t] = est_t
                nc.scalar.activation(
                    out=est_t[:, 0:fill],
                    in_=st_tiles[t][:, 0:fill],
                    func=AF.Exp,
                    scale=0.125,
                )

            def emit_uts(t):
                for bi in ready_by_tile.pop(t, []):
                    b, h, c = blocks[bi]
                    tt, col = block_ref[bi]
                    ut = ut_tiles[(b, h)]
                    voff = (b * KC + c) * 128
                    nc.tensor.matmul(
                        ut[:],
                        vma[:, voff : voff + 128],
                        est_tiles[tt][:, col : col + QH],
                        start=(c == 0),
                        stop=(c == nreals[b] - 1),
                    )
                    if c == nreals[b] - 1:
                        postprocess(b, h)

            def postprocess(b, h):
                ut = ut_tiles.pop((b, h))
                hs = slice(h * QH, (h + 1) * QH)
                is_last = (b, h) == (slot_order[-1], 1)
                den_lo = usbp.tile([64, QH], F32, tag="den_lo")
                osb = osbp.tile([64, QH], BF16, tag="osb")
                if is_last:
                    # tail: skip the numerator copy -- NRECIP reads PSUM
                    den_hi = usbp.tile([64, QH], F32, tag="den_hi")
                    nc.vector.tensor_copy(out=den_hi[:], in_=ut[64:128, :])
                    nc.gpsimd.dma_start(out=den_lo[:], in_=den_hi[:])
                    num = ut[0:64, :]
                else:
                    usb = usbp.tile([128, QH], F32, tag="usb")
                    nc.vector.tensor_copy(out=usb[:], in_=ut[:])
                    nc.gpsimd.dma_start(out=den_lo[:], in_=usb[64:128, :])
                    num = usb[0:64, :]
                if _HAVE_DVE_EXP:
                    nc.vector._custom_dve(
                        _NRECIP_MUL,
                        out=osb[:],
                        in0=den_lo[:],
                        in1=num,
                        s0=_RC0,
                        s1=_RC1,
                    )
                else:
                    rec = usbp.tile([64, QH], F32, tag="rec")
                    nc.vector.reciprocal_approx_fast(rec[:], den_lo[:])
                    nc.vector.tensor_tensor(
                        out=osb[:], in0=num, in1=rec[:], op=ALU.mult
                    )
                nc.gpsimd.dma_start(out=ot[b, :, hs], in_=osb[:])

            # iterate the stream pair-wise (chunk pairs pack on PE)
            bi = 0
            for b in slot_order:
                nreal = nreals[b]
                for h in (0, 1):
                    if (b, h) not in ut_tiles:
                        ut_tiles[(b, h)] = utp.tile([128, QH], F32, tag="ut", name="ut")
                    hs = slice(h * QH, (h + 1) * QH)
                    npairs = (nreal + 1) // 2
                    for p in range(npairs):
                        chunks = [2 * p] + ([2 * p + 1] if 2 * p + 1 < nreal else [])
                        completed = []
                        for i, c in enumerate(chunks):
                            t, sub = bi // TB, bi % TB
                            if sub == 0:
                                st_tiles[t] = stp.tile(
                                    [128, TB * QH], F32, tag="st", name="st"
                                )
                            rows = slice(64 * i, 64 * i + 64)
                            nc.tensor.matmul(
                                st_tiles[t][:, sub * QH : (sub + 1) * QH],
                                kt2[rows, b * S + c * 128 : b * S + (c + 1) * 128],
                                qt2[rows, b * S + h * QH : b * S + (h + 1) * QH],
                                start=True,
                                stop=True,
                                tile_position=(64 * i, 0),
                            )
                            block_ref[bi] = (t, sub * QH)
                            ready_by_tile.setdefault(t, []).append(bi)
                            bi += 1
                            if sub == TB - 1:
                                completed.append(t)
                        # emit exp/UTs only after BOTH pair STs (keeps the
                        # pair's two matmuls adjacent in PE's in-order queue)
                        for t in completed:
                            emit_exp(t, TB * QH)
                            # UT lag: emit tile t-1's UTs after exp(t)
                            if t >= 1:
                                emit_uts(t - 1)
                            last_done_tile[0] = t
            # stream end: final partial tile + remaining UTs
            t_last = (bi - 1) // TB
            if last_done_tile[0] < t_last:
                emit_exp(t_last, ((bi - 1) % TB + 1) * QH)
            for t in sorted(ready_by_tile.keys()):
                emit_uts(t)

    nc.compile()
    return nc


def _plan(valid_lens):
    """Sort batches by length, deal to (slot, core); per-slot chunk counts."""
    order = np.argsort(-valid_lens, kind="stable")  # [B]
    nprocs, nreals = [], []
    for s in range(BPC):
        slot_max = int(valid_lens[order[s * NCORES]])
        nchunks = max(1, -(-slot_max // 128))  # ceil, >= 1
        nprocs.append(nchunks)
        nreals.append(nchunks)
    return order, tuple(nprocs), tuple(nreals)


plan = _plan
build = _build


def make_in_maps(query, key, value, valid_lens, order):
    bf = ml_dtypes.bfloat16
    qt = query.transpose(0, 2, 1)
    kt = key.transpose(0, 2, 1)
    in_maps = []
    for c in range(NCORES):
        idx = [int(order[s * NCORES + c]) for s in range(BPC)]
        vls = valid_lens[idx]  # [BPC]
        # [V | ones], with rows >= vl zeroed (masking folded into V)
        vexts = np.concatenate(
            [value[idx], np.ones((BPC, S, 64), np.float32)], axis=2
        )
        kk = np.arange(S)[None, :]  # [1, S]
        vmask = (kk < vls[:, None]).astype(np.float32)  # [BPC, S]
        vexts *= vmask[:, :, None]
        qd = np.concatenate([qt[idx]] * 2, axis=1)  # [BPC, 128, S]
        kd = np.concatenate([kt[idx]] * 2, axis=1)
        in_maps.append(
            {
                "qt": np.ascontiguousarray(qd).astype(bf),
                "kt": np.ascontiguousarray(kd).astype(bf),
                "v": np.ascontiguousarray(vexts).astype(bf),
            }
        )
    return in_maps


def gather_output(results, order):
    out = np.empty((B, S, D), dtype=np.float32)
    for c in range(NCORES):
        otc = np.asarray(results[c]["ot"]).astype(np.float32)  # [BPC, D, S]
        for s in range(BPC):
            out[int(order[s * NCORES + c])] = otc[s].T
    return out


def kernel(query, key, value, valid_lens):
    query = np.ascontiguousarray(np.asarray(query, dtype=np.float32))
    key = np.ascontiguousarray(np.asarray(key, dtype=np.float32))
    value = np.ascontiguousarray(np.asarray(value, dtype=np.float32))
    valid_lens = np.asarray(valid_lens).astype(np.int32).reshape(B)
    assert query.shape == (B, S, D) and key.shape == (B, S, D)
    assert value.shape == (B, S, D)

    order, nprocs, nreals = _plan(valid_lens)
    cache_key = (nprocs, nreals)
    nc = _BUILD_CACHE.get(cache_key)
    if nc is None:
        nc = _build(nprocs, nreals)
        _BUILD_CACHE[cache_key] = nc

    in_maps = make_in_maps(query, key, value, valid_lens, order)
    res = bass_utils.run_bass_kernel_spmd(nc, in_maps, core_ids=list(range(NCORES)))
    return gather_output(res.results, order)


# revision 17
# speedup vs baseline: 1.0132x; 1.0132x over previous
"""Trainium2 Bass kernel for nn_DotProductAttention_6030134084023.

reference: softmax(mask(Q @ K^T / sqrt(64), valid_lens)) @ V
  query/key/value: [64, 1024, 64] f32, valid_lens: [64] int32 -> [64, 1024, 64] f32

Strategy (v3)
-------------
Batch dim sharded across the 8 NeuronCores; host sorts batches by valid_len
(descending) and deals them round-robin so slot s holds similar lengths on
every core. Compiled per valid_lens chunk-count pattern.

All matmuls bf16 (fp32 PSUM); host pre-casts/transposes and duplicates Q/K
across both SBUF partition halves (the ST pair packing reads chunk A from
partitions 0:64 and chunk B from 64:128). Masking is folded into V on the
host: V rows (and the appended ones-columns) are zeroed for keys >= vl, so
invalid keys contribute exactly 0 to both numerator and denominator -- the
kernel computes exp with NO mask/bias anywhere, which lets exp ops fuse
freely across chunk/slot boundaries.

Each slot is processed in two query passes (h0: q 0:512, h1: 512:1024) so
the UT accumulator is [128,512] = 1 PSUM bank (utp bufs=2 -> no stall at
slot/pass boundaries). The work stream is a flat sequence of 512-col
"blocks" (slot, pass, chunk); blocks land in [128,1536] 3-bank PSUM super-
tiles (stp bufs=2; 6 banks; 8 total with utp):

  ST block  = K_c^T @ Q_h          PE bf16; chunk pairs concurrent via
                                   tile_position row packing (K=64 each)
  exp tile  = exp(0.125 * ST)      ScalarE, ONE fused [128,<=1536] ACTIVATE
                                   per super-tile (3 blocks share the
                                   352-cycle instruction overhead)
  UT_bh    += Vm_c^T @ EST_block   PE bf16 K=128, PSUM-accumulated; Vm_c =
                                   [V_c | ones] so UT rows 64:128 hold the
                                   denominator replicated (lane-local
                                   normalize); UTs deferred one super-tile
                                   to avoid head-of-line blocking STs

The ACT exp table is preloaded at t=0 by a dummy ACTIVATE so the ~1.3us
table load overlaps the input DMA wait. Slot 0's K (first 2 chunks) and
Q h0 are DMA'd first so the pipeline ramps early. Q/K ride the sync DMA
queue; V/den/out ride gpsimd.

Postprocess per (slot, pass): DVE copy UT PSUM->SBUF, DMA the denominator
rows from partitions 64:128 to 0:64 (engines are partition-local; DMA is
the only cross-partition mover), fused custom-DVE osb = num * approx(1/den)
(bit-trick seed + one Newton step), bf16 out DMA. The final pass skips the
numerator copy (NRECIP reads it straight from PSUM) and copies only the
denominator rows, shortening the kernel tail.
"""

import re

import numpy as np
import ml_dtypes

import concourse.bass as bass
import concourse.bacc as bacc
import concourse.tile as tile
from concourse import mybir
from concourse import bass_utils
from concourse import dve_ops as _dve_ops
from concourse.dve_ops import DveOp
from concourse.dve_spec import Spec, Src0, Src1, One, C0, C1, C2, sq

F32 = mybir.dt.float32
BF16 = mybir.dt.bfloat16
I32 = mybir.dt.int32
AF = mybir.ActivationFunctionType
ALU = mybir.AluOpType

NCORES = 8
B = 64
S = 1024
D = 64
BPC = B // NCORES  # 8 batch slots per core
KC = S // 128  # 8 k-chunks of 128
QH = 512  # q-half (one pass)
TB = 3  # ST blocks per PSUM super-tile (3 banks)

DVE_EXP_MOD = 0  # DVE-exp offload disabled for now

_BUILD_CACHE = {}


# ---------------------------------------------------------------------------
# Custom DVE ops (exp offload + fused reciprocal-multiply)
# ---------------------------------------------------------------------------

def _register_op(name, spec):
    """Register a new custom-DVE op at runtime: reserve a free opcode row,
    then compile once to learn (and pin) the uops sha."""
    from concourse.dve_table_gen import free_opcode_rows

    if name in _dve_ops._SUB_OPCODE_FOR_NAME:
        return next(o for o in _dve_ops.OPS if o.name == name)
    row = _dve_ops._CUSTOM_DVE_ROW_BASE + len(_dve_ops.OPS)
    if row >= 0x20 or row not in set(free_opcode_rows("TRN2")):
        raise RuntimeError(f"no free custom-DVE row for {name}")
    op = DveOp(name, spec, subdim=False, uops_sha={})
    _dve_ops.OPS.append(op)
    _dve_ops._SUB_OPCODE_FOR_NAME[name] = row
    try:
        op.compile("v3")
    except ValueError as e:
        m = re.search(r"v3: ([0-9a-f]+)", str(e))
        if not m:
            _dve_ops.OPS.remove(op)
            del _dve_ops._SUB_OPCODE_FOR_NAME[name]
            raise
        op = DveOp(name, spec, subdim=False, uops_sha={"v3": m.group(1)})
        _dve_ops.OPS[-1] = op
    op.compile("v3")
    _dve_ops.CUSTOM_DVE_SPECS[name] = op.spec
    return op


def _bitnot(x):
    from concourse.dve_spec import Bin, AluOp

    return Bin(AluOp.BITWISE_NOT, x, x)


def _np_bitnot(x):
    return (~np.ascontiguousarray(x, np.float32).view(np.int32)).view(np.float32)


# seed constants shared with RECIPROCAL_APPROX_FAST (1-NR variant: ~0.17% err)
_RC0, _RC1 = -0.23549792, 2.0017324

ALPHA = 0.125 / 256.0  # exp arg prescale for the DVE poly path (8 squarings)


def _register_dve_ops():
    try:
        # est = ((b2*s + b1)*s + 1) * mask   (deg-2 poly of exp(s/2048))
        op1 = _register_op(
            "EXP_P2M_ANT",
            Spec(
                body=((C1 * Src0 + C2) * Src0 + One) * C0,
                reference=lambda in0, in1, s0, s1, imm2: (
                    (s1 * in0 + imm2) * in0 + 1.0
                )
                * s0,
            ),
        )
        sq8 = Src0
        for _ in range(8):
            sq8 = sq(sq8)
        op2 = _register_op(
            "EXP_SQ8_ANT",
            Spec(body=sq8, reference=lambda in0, in1, s0, s1, imm2: in0 ** 256),
        )
        # out = in1 / in0 (approx): bit-trick seed + one Newton pass, fused mult
        _y0 = _bitnot(Src0) * C0
        op3 = _register_op(
            "NRECIP_MUL_ANT",
            Spec(
                body=(_y0 * (C1 - Src0 * _y0)) * Src1,
                reference=lambda in0, in1, s0, s1, imm2: (
                    (_np_bitnot(in0) * s0)
                    * (s1 - in0 * (_np_bitnot(in0) * s0))
                )
                * in1,
            ),
        )
        return op1, op2, op3
    except Exception:
        return None


_DVE_OPS = _register_dve_ops()
_HAVE_DVE_EXP = _DVE_OPS is not None
if _DVE_OPS:
    _EXP_P2M, _EXP_SQ8, _NRECIP_MUL = _DVE_OPS
_B1 = ALPHA
_B2 = ALPHA * ALPHA / 2.0


def _build(nprocs, nreals, debug_dump=False, ncores=NCORES):
    """nreals[s]: number of 128-key chunks with any valid key for slot s."""
    nc = bacc.Bacc("TRN2", target_bir_lowering=False, debug=False, num_devices=ncores)
    qt = nc.dram_tensor("qt", [BPC, D, S], BF16, kind="ExternalInput").ap()
    kt = nc.dram_tensor("kt", [BPC, D, S], BF16, kind="ExternalInput").ap()
    # v is [V | ones] with rows >= vl zeroed (masking folded into V)
    v = nc.dram_tensor("v", [BPC, S, 128], BF16, kind="ExternalInput").ap()
    ot = nc.dram_tensor("ot", [BPC, D, S], BF16, kind="ExternalOutput").ap()

    # Long slot FIRST (the ramp needs deep ST lookahead before PE hits
    # exp-dependent UTs) and the LONGEST slot LAST (its h0 postprocess
    # chain hides under its own h1 blocks; only h1's short chain trails).
    slot_order = [1, 6, 7, 4, 2, 5, 3, 0]

    # flat block stream: (slot, pass, chunk); pairs of chunks pack on PE
    blocks = []  # (b, h, c)
    for b in slot_order:
        for h in (0, 1):
            for c in range(nreals[b]):
                blocks.append((b, h, c))
    nblocks = len(blocks)
    ntiles = -(-nblocks // TB)

    with tile.TileContext(nc) as tc:
        with (
            tc.tile_pool(name="const", bufs=1) as constp,
            tc.tile_pool(name="est", bufs=5) as estp,
            tc.tile_pool(name="usb", bufs=4) as usbp,
            tc.tile_pool(name="osb", bufs=3) as osbp,
            tc.tile_pool(name="stp", bufs=2, space="PSUM") as stp,
            tc.tile_pool(name="utp", bufs=2, space="PSUM") as utp,
        ):
            # ---- persistent SBUF inputs ----
            qt2 = constp.tile([128, BPC * S], BF16, tag="qt2")
            kt2 = constp.tile([128, BPC * S], BF16, tag="kt2")
            # vma[p, (b*KC + kc)*128 + j]: j<64 -> V[b, kc*128+p, j]; j>=64 ->
            # 1.0 (or 0.0 where kc*128+p >= vl[b])
            vma = constp.tile([128, BPC * KC * 128], BF16, tag="vma")
            vview = vma[:].rearrange("p (b j) -> p b j", j=128)

            # Input DMAs: per-partition-half pieces spread across the three
            # DMA-capable queues (sync/gpsimd/scalar) so transfers land in
            # parallel. The ramp-critical first-slot pieces go first: K
            # halves on sync+gpsimd, Q(cols 0:QH) half0 as the scalar
            # queue's first instruction (its stream is idle until ~10.5us).
            h0, h1 = slice(0, 64), slice(64, 128)
            b0 = slot_order[0]
            kwb0 = nreals[b0] * 128
            ksp = min(512, kwb0)
            nc.scalar.dma_start(
                out=qt2[h0, b0 * S : b0 * S + QH], in_=qt[b0, :, 0:QH]
            )
            nc.sync.dma_start(
                out=kt2[h0, b0 * S : b0 * S + ksp], in_=kt[b0, :, 0:ksp]
            )
            nc.gpsimd.dma_start(
                out=kt2[h1, b0 * S : b0 * S + ksp], in_=kt[b0, :, 0:ksp]
            )
            nc.sync.dma_start(
                out=qt2[h1, b0 * S : b0 * S + QH], in_=qt[b0, :, 0:QH]
            )

            # ---- ACT exp-table preload: dummy ACTIVATE with no DMA deps ----
            warm = constp.tile([128, 1], F32, tag="warm")
            nc.gpsimd.memset(warm[:], 0.0)
            wout = constp.tile([128, 1], F32, tag="wout")
            nc.scalar.activation(out=wout[:], in_=warm[:], func=AF.Exp, scale=0.125)

            # first slot's remaining pieces, then the other slots
            if kwb0 > ksp:
                nc.sync.dma_start(
                    out=kt2[h0, b0 * S + ksp : b0 * S + kwb0],
                    in_=kt[b0, :, ksp:kwb0],
                )
                nc.gpsimd.dma_start(
                    out=kt2[h1, b0 * S + ksp : b0 * S + kwb0],
                    in_=kt[b0, :, ksp:kwb0],
                )
            nc.sync.dma_start(
                out=qt2[h0, b0 * S + QH : b0 * S + S], in_=qt[b0, :, QH:S]
            )
            nc.gpsimd.dma_start(
                out=qt2[h1, b0 * S + QH : b0 * S + S], in_=qt[b0, :, QH:S]
            )

            def vdma(queue, b):
                queue.dma_start(
                    out=vview[:, b * KC : b * KC + nreals[b], :],
                    in_=bass.AP(
                        tensor=v.tensor,
                        offset=v[b].offset,
                        ap=[[128, 128], [128 * 128, nreals[b]], [1, 128]],
                    ),
                )

            vdma(nc.scalar, b0)
            for vi, b in enumerate(slot_order[1:]):
                kwb = nreals[b] * 128
                nc.sync.dma_start(out=qt2[h0, b * S : (b + 1) * S], in_=qt[b])
                nc.gpsimd.dma_start(out=qt2[h1, b * S : (b + 1) * S], in_=qt[b])
                nc.sync.dma_start(
                    out=kt2[h0, b * S : b * S + kwb], in_=kt[b, :, 0:kwb]
                )
                nc.gpsimd.dma_start(
                    out=kt2[h1, b * S : b * S + kwb], in_=kt[b, :, 0:kwb]
                )
                vdma(nc.gpsimd if vi % 2 == 0 else nc.sync, b)

            # ---- main stream ----
            # variable tile sizes: [1, 2, 3, 3, ...] -- the first exp fires
            # after a single ST block (fast ramp); steady state fuses 3.
            tile_sizes = [1, 2] if nblocks > 3 else [nblocks]
            rem = nblocks - sum(tile_sizes)
            while rem > 0:
                s = min(TB, rem)
                tile_sizes.append(s)
                rem -= s
            tile_of_block, sub_of_block = [], []
            for ti, sz in enumerate(tile_sizes):
                for s in range(sz):
                    tile_of_block.append(ti)
                    sub_of_block.append(s)

            st_tiles = {}  # tile idx -> psum tile
            est_tiles = {}  # tile idx -> est sbuf tile
            ut_tiles = {}  # (b, h) -> psum tile
            block_ref = {}  # block idx -> (tile idx, col)
            ready_by_tile = {}  # tile idx -> [block idx] fully est'd

            def emit_exp(t):
                fill = tile_sizes[t] * QH
                est_t = estp.tile([128, TB * QH], BF16, tag="est", name="est")
                est_tiles[t] = est_t
                nc.scalar.activation(
                    out=est_t[:, 0:fill],
                    in_=st_tiles[t][:, 0:fill],
                    func=AF.Exp,
                    scale=0.125,
                )

            def emit_uts(t):
                for bi in ready_by_tile.pop(t, []):
                    b, h, c = blocks[bi]
                    tt, col = block_ref[bi]
                    ut = ut_tiles[(b, h)]
                    voff = (b * KC + c) * 128
                    nc.tensor.matmul(
                        ut[:],
                        vma[:, voff : voff + 128],
                        est_tiles[tt][:, col : col + QH],
                        start=(c == 0),
                        stop=(c == nreals[b] - 1),
                    )
                    if c == nreals[b] - 1:
                        postprocess(b, h)

            def recip_mul(out_ap, den_ap, num_ap):
                if _HAVE_DVE_EXP:
                    nc.vector._custom_dve(
                        _NRECIP_MUL, out=out_ap, in0=den_ap, in1=num_ap,
                        s0=_RC0, s1=_RC1,
                    )
                else:
                    rec = usbp.tile([64, QH], F32, tag="rec", name="rec")
                    r = rec[:, 0 : den_ap.shape[-1]]
                    nc.vector.reciprocal_approx_fast(r, den_ap)
                    nc.vector.tensor_tensor(out=out_ap, in0=num_ap, in1=r, op=ALU.mult)

            def postprocess(b, h):
                # den DMAs ride the sync queue (idle once inputs land) so an
                # out-DMA waiting on its NRECIP never blocks the next pass's
                # den issue; out DMAs ride gpsimd.
                ut = ut_tiles.pop((b, h))
                hs = slice(h * QH, (h + 1) * QH)
                tail = b == slot_order[-1]
                den_lo = usbp.tile([64, QH], F32, tag="den_lo")
                osb = osbp.tile([64, QH], BF16, tag="osb")
                if tail:
                    # tail-slot passes: skip the numerator copy (NRECIP reads
                    # it straight from PSUM -- nothing reuses these banks) and
                    # pipeline the chain in 256-col pieces. Emission order is
                    # copies, DMAs, NRECIPs, outs: a NRECIP stalled on its den
                    # DMA must never sit ahead of a copy in DVE's in-order
                    # queue.
                    den_hi = usbp.tile([64, QH], F32, tag="den_hi")
                    pieces = [slice(j * 256, (j + 1) * 256) for j in (0, 1)]
                    for qs in pieces:
                        nc.vector.tensor_copy(
                            out=den_hi[:, qs], in_=ut[64:128, qs]
                        )
                    for qs in pieces:
                        nc.sync.dma_start(out=den_lo[:, qs], in_=den_hi[:, qs])
                    for qs in pieces:
                        recip_mul(osb[:, qs], den_lo[:, qs], ut[0:64, qs])
                    # single out DMA: [64,512] writes 1KB DRAM runs; 256-col
                    # pieces would write 512B runs with ~2x completion latency
                    nc.gpsimd.dma_start(out=ot[b, :, hs], in_=osb[:])
                    return
                usb = usbp.tile([128, QH], F32, tag="usb")
                nc.vector.tensor_copy(out=usb[:], in_=ut[:])
                nc.sync.dma_start(out=den_lo[:], in_=usb[64:128, :])
                recip_mul(osb[:], den_lo[:], usb[0:64, :])
                nc.gpsimd.dma_start(out=ot[b, :, hs], in_=osb[:])

            # iterate the stream pair-wise (chunk pairs pack on PE)
            bi = 0
            for b in slot_order:
                nreal = nreals[b]
                for h in (0, 1):
                    if (b, h) not in ut_tiles:
                        ut_tiles[(b, h)] = utp.tile([128, QH], F32, tag="ut", name="ut")
                    hs = slice(h * QH, (h + 1) * QH)
                    npairs = (nreal + 1) // 2
                    for p in range(npairs):
                        chunks = [2 * p] + ([2 * p + 1] if 2 * p + 1 < nreal else [])
                        completed = []
                        for i, c in enumerate(chunks):
                            t, sub = tile_of_block[bi], sub_of_block[bi]
                            if sub == 0:
                                st_tiles[t] = stp.tile(
                                    [128, TB * QH], F32, tag="st", name="st"
                                )
                            rows = slice(64 * i, 64 * i + 64)
                            nc.tensor.matmul(
                                st_tiles[t][:, sub * QH : (sub + 1) * QH],
                                kt2[rows, b * S + c * 128 : b * S + (c + 1) * 128],
                                qt2[rows, b * S + h * QH : b * S + (h + 1) * QH],
                                start=True,
                                stop=True,
                                tile_position=(64 * i, 0),
                            )
                            block_ref[bi] = (t, sub * QH)
                            ready_by_tile.setdefault(t, []).append(bi)
                            bi += 1
                            if sub == tile_sizes[t] - 1:
                                completed.append(t)
                        # emit exp/UTs only after BOTH pair STs (keeps the
                        # pair's two matmuls adjacent in PE's in-order queue)
                        for t in completed:
                            emit_exp(t)
                            # UT lag: emit tile t-1's UTs after exp(t)
                            if t >= 1:
                                emit_uts(t - 1)
            # stream end: remaining UTs (the last tile always completes in
            # the loop since tile_sizes sums to nblocks)
            for t in sorted(ready_by_tile.keys()):
                emit_uts(t)

    nc.compile()
    return nc


def _plan(valid_lens):
    """Sort batches by length, deal to (slot, core); per-slot chunk counts."""
    order = np.argsort(-valid_lens, kind="stable")  # [B]
    nprocs, nreals = [], []
    for s in range(BPC):
        slot_max = int(valid_lens[order[s * NCORES]])
        nchunks = max(1, -(-slot_max // 128))  # ceil, >= 1
        nprocs.append(nchunks)
        nreals.append(nchunks)
    return order, tuple(nprocs), tuple(nreals)


plan = _plan
build = _build


def make_in_maps(query, key, value, valid_lens, order):
    bf = ml_dtypes.bfloat16
    qt = query.transpose(0, 2, 1)
    kt = key.transpose(0, 2, 1)
    in_maps = []
    for c in range(NCORES):
        idx = [int(order[s * NCORES + c]) for s in range(BPC)]
        vls = valid_lens[idx]  # [BPC]
        # [V | ones], with rows >= vl zeroed (masking folded into V)
        vexts = np.concatenate(
            [value[idx], np.ones((BPC, S, 64), np.float32)], axis=2
        )
        kk = np.arange(S)[None, :]  # [1, S]
        vmask = (kk < vls[:, None]).astype(np.float32)  # [BPC, S]
        vexts *= vmask[:, :, None]
        in_maps.append(
            {
                "qt": np.ascontiguousarray(qt[idx]).astype(bf),
                "kt": np.ascontiguousarray(kt[idx]).astype(bf),
                "v": np.ascontiguousarray(vexts).astype(bf),
            }
        )
    return in_maps


def gather_output(results, order):
    out = np.empty((B, S, D), dtype=np.float32)
    for c in range(NCORES):
        otc = np.asarray(results[c]["ot"]).astype(np.float32)  # [BPC, D, S]
        for s in range(BPC):
            out[int(order[s * NCORES + c])] = otc[s].T
    return out


def kernel(query, key, value, valid_lens):
    query = np.ascontiguousarray(np.asarray(query, dtype=np.float32))
    key = np.ascontiguousarray(np.asarray(key, dtype=np.float32))
    value = np.ascontiguousarray(np.asarray(value, dtype=np.float32))
    valid_lens = np.asarray(valid_lens).astype(np.int32).reshape(B)
    assert query.shape == (B, S, D) and key.shape == (B, S, D)
    assert value.shape == (B, S, D)

    order, nprocs, nreals = _plan(valid_lens)
    cache_key = (nprocs, nreals)
    nc = _BUILD_CACHE.get(cache_key)
    if nc is None:
        nc = _build(nprocs, nreals)
        _BUILD_CACHE[cache_key] = nc

    in_maps = make_in_maps(query, key, value, valid_lens, order)
    res = bass_utils.run_bass_kernel_spmd(nc, in_maps, core_ids=list(range(NCORES)))
    return gather_output(res.results, order)


# revision 24
# speedup vs baseline: 1.0761x; 1.0621x over previous
"""Trainium2 Bass kernel for nn_DotProductAttention_6030134084023.

reference: softmax(mask(Q @ K^T / sqrt(64), valid_lens)) @ V
  query/key/value: [64, 1024, 64] f32, valid_lens: [64] int32 -> [64, 1024, 64] f32

Strategy (v2)
-------------
Batch dim sharded across the 8 NeuronCores; host sorts batches by valid_len
(descending) and deals them round-robin so slot s holds similar lengths on
every core. The kernel is compiled per valid_lens pattern (chunk counts are
specialized; correctness never depends on the specialization since skipped
chunks are exactly-masked).

All matmuls in bf16 (fp32 PSUM accumulation); host pre-casts/transposes
Q,K,V (layout+dtype only) and post-casts the output.

Per-core dataflow per batch slot, in "S^T orientation" (keys on SBUF
partitions, queries on the free dim — no transposes anywhere):

  ST_c[k, q]  = KT_c.T @ QT        PE bf16; chunk pairs run CONCURRENTLY
                                   via tile_position row packing (K=64)
  EST_c       = exp(0.125 ST + m)  ScalarE, one [128,1024] op per chunk;
                                   valid_lens masking folded in as a
                                   per-partition bias AP (0 valid / -80
                                   invalid) -- no separate mask work at all
  UT[:, q]   += Vm_c.T @ EST_c     PE bf16 K=128, PSUM-accumulated, where
                                   Vm_c = [V_c | ones*64] ([128, 128], ones
                                   appended by the host) so UT rows 64..127
                                   hold the softmax denominator replicated
                                   64x (makes the normalize lane-local)

ScalarE's exp stream (1 elem/lane/cycle, ~43us) is the critical path; UT
matmuls are deferred one pair so an est-waiting UT never head-of-line
blocks STs in PE's in-order queue. Postprocess per slot: DVE copy UT
PSUM->SBUF (ScalarE copy for the tail slots, where it is idle), DMA the
denominator rows from partitions 64..127 to 0..63 (engines are
partition-local; DMA is the only cross-partition mover), then a single
fused custom-DVE op osb = num * approx(1/den) (bit-trick seed + one
Newton step), bf16 out DMA per q-half.

A custom 2-op DVE exp (deg-2 poly on s/2048, then 8 squarings) was built
and verified on HW, but routing chunks to it measured net-negative (PE
head-of-line stalls starve ScalarE), so exp stays on ScalarE.

v2.1 patches (measured: median 64.8us -> 63.7us, min 62.8us, much tighter
run-to-run spread):
 - ACT exp-table preload: a dummy ACTIVATE with no DMA deps at the top of
   the scalar queue, so the ~1.3us table load overlaps the input-DMA wait
   instead of gating the first real exp.
 - tail-slot postprocess: skip the numerator copy (NRECIP reads it straight
   from PSUM; nothing reuses those banks afterwards), copy only the den
   rows, den DMA on the otherwise-idle scalar HWDGE ring, and emit
   copies -> DMAs -> NRECIPs -> outs so a NRECIP stalled on its den DMA
   never blocks a copy in DVE's in-order queue.
Tried and rejected (slower or no better on HW): fused 3-block exp
ACTIVATEs with pass-split [128,512] UT accumulators (ScalarE busy drops
43.9->39.4us but the stream becomes PE-paced with zero slack -- every
stall/downclock stretches it); fp8 e4m3 est x V for DoubleRow UT (rel err
3.7e-2 > 2e-2 gate, TRN e4m3 exp overflow at 240); 1024-col matmuls into
2-bank PSUM (NEFF compile rejects); multi-queue input-DMA spread (slower).
"""

import re

import numpy as np
import ml_dtypes

import concourse.bass as bass
import concourse.bacc as bacc
import concourse.tile as tile
from concourse import mybir
from concourse import bass_utils
from concourse import dve_ops as _dve_ops
from concourse.dve_ops import DveOp
from concourse.dve_spec import Spec, Src0, Src1, One, C0, C1, C2, sq

F32 = mybir.dt.float32
BF16 = mybir.dt.bfloat16
I32 = mybir.dt.int32
AF = mybir.ActivationFunctionType
ALU = mybir.AluOpType

NCORES = 8
B = 64
S = 1024
D = 64
BPC = B // NCORES  # 8 batch slots per core
KC = S // 128  # 8 k-chunks of 128
QH = 512  # q-half

NEG_BIAS = -80.0  # exp(0.125*s - 80) ~ 0 for any |s| <= 50
ALPHA = 0.125 / 256.0  # exp arg prescale for the DVE poly path (8 squarings)
DVE_EXP_MOD = 0  # DVE-exp offload disabled (measured net-negative)

_BUILD_CACHE = {}


# ---------------------------------------------------------------------------
# Custom DVE exp: est = ((b2*s + b1)*s + 1)^256 * mask  ~=  exp(0.125*s)*mask
# op1: p = ((C1*s + C2)*s + One) * C0   (C0 = per-partition 0/1 mask)
# op2: p^256 via 8 squarings
# ---------------------------------------------------------------------------

def _register_op(name, spec):
    """Register a new custom-DVE op at runtime: reserve a free opcode row,
    then compile once to learn (and pin) the uops sha."""
    from concourse.dve_table_gen import free_opcode_rows

    if name in _dve_ops._SUB_OPCODE_FOR_NAME:
        return next(o for o in _dve_ops.OPS if o.name == name)
    row = _dve_ops._CUSTOM_DVE_ROW_BASE + len(_dve_ops.OPS)
    if row >= 0x20 or row not in set(free_opcode_rows("TRN2")):
        raise RuntimeError(f"no free custom-DVE row for {name}")
    op = DveOp(name, spec, subdim=False, uops_sha={})
    _dve_ops.OPS.append(op)
    _dve_ops._SUB_OPCODE_FOR_NAME[name] = row
    try:
        op.compile("v3")
    except ValueError as e:
        m = re.search(r"v3: ([0-9a-f]+)", str(e))
        if not m:
            _dve_ops.OPS.remove(op)
            del _dve_ops._SUB_OPCODE_FOR_NAME[name]
            raise
        op = DveOp(name, spec, subdim=False, uops_sha={"v3": m.group(1)})
        _dve_ops.OPS[-1] = op
    op.compile("v3")
    _dve_ops.CUSTOM_DVE_SPECS[name] = op.spec
    return op


def _bitnot(x):
    from concourse.dve_spec import Bin, AluOp

    return Bin(AluOp.BITWISE_NOT, x, x)


def _np_bitnot(x):
    return (~np.ascontiguousarray(x, np.float32).view(np.int32)).view(np.float32)


# seed constants shared with RECIPROCAL_APPROX_FAST (1-NR variant: ~0.17% err)
_RC0, _RC1 = -0.23549792, 2.0017324


def _register_dve_ops():
    try:
        # est = ((b2*s + b1)*s + 1) * mask   (deg-2 poly of exp(s/2048))
        op1 = _register_op(
            "EXP_P2M_ANT",
            Spec(
                body=((C1 * Src0 + C2) * Src0 + One) * C0,
                reference=lambda in0, in1, s0, s1, imm2: (
                    (s1 * in0 + imm2) * in0 + 1.0
                )
                * s0,
            ),
        )
        sq8 = Src0
        for _ in range(8):
            sq8 = sq(sq8)
        op2 = _register_op(
            "EXP_SQ8_ANT",
            Spec(body=sq8, reference=lambda in0, in1, s0, s1, imm2: in0 ** 256),
        )
        # out = in1 / in0 (approx): bit-trick seed + one Newton pass, fused mult
        _y0 = _bitnot(Src0) * C0
        op3 = _register_op(
            "NRECIP_MUL_ANT",
            Spec(
                body=(_y0 * (C1 - Src0 * _y0)) * Src1,
                reference=lambda in0, in1, s0, s1, imm2: (
                    (_np_bitnot(in0) * s0)
                    * (s1 - in0 * (_np_bitnot(in0) * s0))
                )
                * in1,
            ),
        )
        return op1, op2, op3
    except Exception:
        return None


_DVE_OPS = _register_dve_ops()
_HAVE_DVE_EXP = _DVE_OPS is not None
if _DVE_OPS:
    _EXP_P2M, _EXP_SQ8, _NRECIP_MUL = _DVE_OPS
_B1 = ALPHA
_B2 = ALPHA * ALPHA / 2.0


def _build(nprocs, nreals, debug_dump=False, ncores=NCORES):
    """nreals[s]: number of 128-key chunks with any valid key for slot s."""
    nc = bacc.Bacc("TRN2", target_bir_lowering=False, debug=False, num_devices=ncores)
    qt = nc.dram_tensor("qt", [BPC, D, S], BF16, kind="ExternalInput").ap()
    kt = nc.dram_tensor("kt", [BPC, D, S], BF16, kind="ExternalInput").ap()
    # v is [V | ones]: host appends 64 ones columns so UT rows 64..127
    # accumulate the softmax denominator (replicated for lane-local normalize)
    v = nc.dram_tensor("v", [BPC, S, 128], BF16, kind="ExternalInput").ap()
    bias_t = nc.dram_tensor("bias_t", [128, KC * BPC], F32, kind="ExternalInput").ap()
    mask_t = nc.dram_tensor("mask_t", [128, KC * BPC], F32, kind="ExternalInput").ap()
    ot = nc.dram_tensor("ot", [BPC, D, S], BF16, kind="ExternalOutput").ap()
    usb_o = None
    if debug_dump:
        usb_o = nc.dram_tensor(
            "usb_o", [BPC, 128, S], F32, kind="ExternalOutput"
        ).ap()

    use_dve = _HAVE_DVE_EXP and DVE_EXP_MOD > 0

    with tile.TileContext(nc) as tc:
        with (
            tc.tile_pool(name="const", bufs=1) as constp,
            tc.tile_pool(name="pt", bufs=2) as ptp,
            tc.tile_pool(name="est", bufs=8) as estp,
            tc.tile_pool(name="usb", bufs=4) as usbp,
            tc.tile_pool(name="osb", bufs=2) as osbp,
            tc.tile_pool(name="stp", bufs=3, space="PSUM") as stp,
            tc.tile_pool(name="utp", bufs=1, space="PSUM") as utp,
        ):
            # ---- ACT exp-table preload: dummy ACTIVATE with no DMA deps
            # (the ~1.3us table load overlaps the input-DMA wait) ----
            warm = constp.tile([128, 1], F32, tag="warm")
            nc.gpsimd.memset(warm[:], 0.0)
            wout = constp.tile([128, 1], F32, tag="wout")
            nc.scalar.activation(out=wout[:], in_=warm[:], func=AF.Exp, scale=0.125)

            # ---- persistent SBUF inputs ----
            qt2 = constp.tile([128, BPC * S], BF16, tag="qt2")
            kt2 = constp.tile([128, BPC * S], BF16, tag="kt2")
            # vma[p, (b*KC + kc)*128 + j]: j<64 -> V[b, kc*128+p, j]; j>=64 -> 1.0
            vma = constp.tile([128, BPC * KC * 128], BF16, tag="vma")
            bias_sb = constp.tile([128, KC * BPC], F32, tag="bias")
            mask_sb = constp.tile([128, KC * BPC], F32, tag="mask")

            nc.gpsimd.dma_start(out=bias_sb[:], in_=bias_t)
            nc.gpsimd.dma_start(out=mask_sb[:], in_=mask_t)

            vview = vma[:].rearrange("p (b j) -> p b j", j=128)

            # Q/K on sync queue, V on gpsimd queue, both in slot_order-first
            # groups so the first slot's inputs land before the rest.
            # Longest slot FIRST: the pipeline ramp needs deep ST lookahead
            # before PE hits exp-dependent UT matmuls (short-slot-first
            # measured a 3.7us ScalarE stall); end with one long-ish slot so
            # only a single postprocess chain trails the exp stream.
            slot_order = [0, 6, 7, 1, 4, 2, 5, 3]
            dma_groups = [slot_order[0:1]] + [
                slot_order[1 + 2 * g : 3 + 2 * g] for g in range(4)
            ]
            for grp in dma_groups:
                for b in grp:
                    for half in (slice(0, 64), slice(64, 128)):
                        nc.sync.dma_start(
                            out=qt2[half, b * S : (b + 1) * S], in_=qt[b]
                        )
                        kwb = nreals[b] * 128
                        nc.sync.dma_start(
                            out=kt2[half, b * S : b * S + kwb], in_=kt[b, :, 0:kwb]
                        )
                    nc.gpsimd.dma_start(
                        out=vview[:, b * KC : b * KC + nreals[b], :],
                        in_=bass.AP(
                            tensor=v.tensor,
                            offset=v[b].offset,
                            ap=[[128, 128], [128 * 128, nreals[b]], [1, 128]],
                        ),
                    )

            # Offloading chunk exps to the custom 2-op DVE exp was measured
            # net-negative in BOTH deferral schemes (1-2 pair lag AND
            # slot-end lag): the DVE chain latency still delays the slot's
            # UT completion / postprocess and starves ScalarE via PSUM
            # buffer back-pressure. All exp stays on ScalarE.
            dve_set = set()

            def exp_chunk(est_t, st_t, b, kc):
                col = kc * BPC + b
                on_dve = (b, kc) in dve_set
                if on_dve:
                    p4 = ptp.tile([128, 2 * QH], F32, tag="p4")
                    nc.vector._custom_dve(
                        _EXP_P2M,
                        out=p4[:],
                        in0=st_t[:],
                        s0=mask_sb[:, col : col + 1],
                        s1=_B2,
                        imm2=_B1,
                    )
                    nc.vector._custom_dve(_EXP_SQ8, out=est_t[:], in0=p4[:])
                else:
                    nc.scalar.activation(
                        out=est_t[:],
                        in_=st_t[:],
                        func=AF.Exp,
                        scale=0.125,
                        bias=bias_sb[:, col : col + 1],
                    )

            for b in slot_order:
                nreal = nreals[b]
                # single [128, 2*QH] accumulator: h=0 in cols 0:QH, h=1 in
                # cols QH:2QH; rows 64..127 hold the replicated denominator
                ut = utp.tile([128, 2 * QH], F32, tag="ut")

                qb = qt2[:, b * S : (b + 1) * S]
                kb = kt2[:, b * S : (b + 1) * S]

                # pipeline: STs run ahead; exp per chunk; UTs deferred one
                # pair (two for slower DVE-exp'd chunks) so an est-waiting UT
                # doesn't head-of-line-block STs in PE's in-order queue
                pend_ut = []  # [(kc, est_t, min_pair)]

                def emit_uts(chunks, nreal=nreal, b=b, ut=ut):
                    for kc, est_t, _ in chunks:
                        voff = (b * KC + kc) * 128
                        for h in (0, 1):
                            nc.tensor.matmul(
                                ut[:, h * QH : (h + 1) * QH],
                                vma[:, voff : voff + 128],
                                est_t[:, h * QH : (h + 1) * QH],
                                start=(kc == 0),
                                stop=(kc == nreal - 1),
                            )

                npairs = (nreal + 1) // 2
                for p in range(npairs):
                    A, Bc = 2 * p, 2 * p + 1
                    chunks = [A] + ([Bc] if Bc < nreal else [])
                    sts = {}
                    for i, c in enumerate(chunks):
                        sts[c] = stp.tile([128, 2 * QH], F32, tag="st", name="st")
                    # ST matmuls: pair concurrent via row packing
                    for h in (0, 1):
                        hs = slice(h * QH, (h + 1) * QH)
                        for i, c in enumerate(chunks):
                            rows = slice(64 * i, 64 * i + 64)
                            nc.tensor.matmul(
                                sts[c][:, hs],
                                kb[rows, c * 128 : (c + 1) * 128],
                                qb[rows, hs],
                                start=True,
                                stop=True,
                                tile_position=(64 * i, 0),
                            )
                    # emit UTs that are due (ACT chunks after 1 pair, DVE
                    # chunks after 2); out-of-kc-order is fine: accumulation
                    # is commutative and kc==0/nreal-1 are never deferred
                    due = [e for e in pend_ut if e[2] <= p]
                    if due:
                        emit_uts(due)
                        pend_ut = [e for e in pend_ut if e[2] > p]
                    # exp this pair's chunks
                    for c in chunks:
                        est_t = estp.tile([128, 2 * QH], BF16, tag="est")
                        exp_chunk(est_t, sts[c], b, c)
                        # DVE chunks flush only at slot end (lag=inf)
                        lag = 99 if (b, c) in dve_set else 1
                        pend_ut.append((c, est_t, p + lag))
                emit_uts(sorted(pend_ut))

                # ---- postprocess: normalize by the replicated denominator ----
                den_lo = usbp.tile([64, 2 * QH], F32, tag="den_lo")
                osb = osbp.tile([64, 2 * QH], BF16, tag="osb")
                hss = [slice(h * QH, (h + 1) * QH) for h in (0, 1)]
                if b == slot_order[-1] and _HAVE_DVE_EXP:
                    # tail slot: skip the numerator copy entirely (NRECIP
                    # reads it straight from PSUM; nothing reuses the banks
                    # afterwards); copy only the den rows, DMA them down on
                    # the idle scalar HWDGE ring. Emission order is copies,
                    # DMAs, NRECIPs, outs so a NRECIP stalled on its den DMA
                    # never blocks a copy in DVE's in-order queue.
                    den_hi = usbp.tile([64, 2 * QH], F32, tag="den_hi")
                    for hs in hss:
                        nc.vector.tensor_copy(
                            out=den_hi[:, hs], in_=ut[64:128, hs]
                        )
                    for hs in hss:
                        nc.scalar.dma_start(out=den_lo[:, hs], in_=den_hi[:, hs])
                    for hs in hss:
                        nc.vector._custom_dve(
                            _NRECIP_MUL,
                            out=osb[:, hs],
                            in0=den_lo[:, hs],
                            in1=ut[0:64, hs],
                            s0=_RC0,
                            s1=_RC1,
                        )
                    for hs in hss:
                        nc.sync.dma_start(out=ot[b, :, hs], in_=osb[:, hs])
                elif _HAVE_DVE_EXP:
                    usb = usbp.tile([128, 2 * QH], F32, tag="usb")
                    nc.vector.tensor_copy(out=usb[:], in_=ut[:])
                    if usb_o is not None:
                        nc.sync.dma_start(out=usb_o[b], in_=usb[:])
                    nc.sync.dma_start(out=den_lo[:], in_=usb[64:128, :])
                    # fused osb = num * ~(1/den) (bit-trick seed + 1 Newton);
                    # two half-ops so the first out-DMA can start earlier
                    for h in (0, 1):
                        hs = slice(h * QH, (h + 1) * QH)
                        nc.vector._custom_dve(
                            _NRECIP_MUL,
                            out=osb[:, hs],
                            in0=den_lo[:, hs],
                            in1=usb[0:64, hs],
                            s0=_RC0,
                            s1=_RC1,
                        )
                        nc.sync.dma_start(
                            out=ot[b, :, hs], in_=osb[:, hs]
                        )
                else:
                    usb = usbp.tile([128, 2 * QH], F32, tag="usb")
                    nc.vector.tensor_copy(out=usb[:], in_=ut[:])
                    nc.sync.dma_start(out=den_lo[:], in_=usb[64:128, :])
                    rec = usbp.tile([64, 2 * QH], F32, tag="rec")
                    nc.vector.reciprocal_approx_fast(rec[:], den_lo[:])
                    nc.gpsimd.tensor_tensor(
                        out=osb[:], in0=usb[0:64, :], in1=rec[:], op=ALU.mult
                    )
                    nc.gpsimd.dma_start(out=ot[b], in_=osb[:])

    nc.compile()
    return nc


def _plan(valid_lens):
    """Sort batches by length, deal to (slot, core); per-slot chunk counts."""
    order = np.argsort(-valid_lens, kind="stable")  # [B]
    nprocs, nreals = [], []
    for s in range(BPC):
        slot_max = int(valid_lens[order[s * NCORES]])
        nchunks = max(1, -(-slot_max // 128))  # ceil, >= 1
        nprocs.append(nchunks)
        nreals.append(nchunks)
    return order, tuple(nprocs), tuple(nreals)


plan = _plan
build = _build


def make_in_maps(query, key, value, valid_lens, order):
    bf = ml_dtypes.bfloat16
    qt = query.transpose(0, 2, 1)
    kt = key.transpose(0, 2, 1)
    iota = np.arange(128)
    in_maps = []
    for c in range(NCORES):
        idx = [int(order[s * NCORES + c]) for s in range(BPC)]
        vls = valid_lens[idx]  # [BPC]
        # mask[p, kc*BPC + b] = 1.0 if kc*128 + p < vl[b] else 0.0
        kk = (128 * np.arange(KC)[:, None, None] + iota[None, None, :])  # [KC,1,128]
        m = (kk < vls[None, :, None]).astype(np.float32)  # [KC, BPC, 128]
        mask_t = np.ascontiguousarray(
            m.transpose(2, 0, 1).reshape(128, KC * BPC)
        )
        bias_t = (mask_t - 1.0) * (-NEG_BIAS)  # 0 valid, -80 invalid
        vexts = np.concatenate(
            [value[idx], np.ones((BPC, S, 64), np.float32)], axis=2
        )
        in_maps.append(
            {
                "qt": np.ascontiguousarray(qt[idx]).astype(bf),
                "kt": np.ascontiguousarray(kt[idx]).astype(bf),
                "v": np.ascontiguousarray(vexts).astype(bf),
                "bias_t": np.ascontiguousarray(bias_t.astype(np.float32)),
                "mask_t": mask_t,
            }
        )
    return in_maps


def gather_output(results, order):
    out = np.empty((B, S, D), dtype=np.float32)
    for c in range(NCORES):
        otc = np.asarray(results[c]["ot"]).astype(np.float32)  # [BPC, D, S]
        for s in range(BPC):
            out[int(order[s * NCORES + c])] = otc[s].T
    return out


def kernel(query, key, value, valid_lens):
    query = np.ascontiguousarray(np.asarray(query, dtype=np.float32))
    key = np.ascontiguousarray(np.asarray(key, dtype=np.float32))
    value = np.ascontiguousarray(np.asarray(value, dtype=np.float32))
    valid_lens = np.asarray(valid_lens).astype(np.int32).reshape(B)
    assert query.shape == (B, S, D) and key.shape == (B, S, D)
    assert value.shape == (B, S, D)

    order, nprocs, nreals = _plan(valid_lens)
    cache_key = (nprocs, nreals)
    nc = _BUILD_CACHE.get(cache_key)
    if nc is None:
        nc = _build(nprocs, nreals)
        _BUILD_CACHE[cache_key] = nc

    in_maps = make_in_maps(query, key, value, valid_lens, order)
    res = bass_utils.run_bass_kernel_spmd(nc, in_maps, core_ids=list(range(NCORES)))
    return gather_output(res.results, order)

